# revision 36
# baseline (speedup 1.0000x reference)
"""GumbelSelector Trainium2 kernel.

Math: h = relu(s @ W1 + b1); lo = h @ W2 + b2  (2 classes)
  dec  = (argmax(lo) == 1)  ==  (z > 0)         where z = h @ (W2[:,1]-W2[:,0]) + (b2[1]-b2[0])
  prob = softmax(lo)[..., 1] ==  sigmoid(z)
  Per-row correction (LB=1): if a row of dec is all zero, activate argmax(rnoise).

Sharding: data-parallel over batch B=64 -> 8 cores x 8 rows. Weights replicated.
Host pre-transposes each core's s shard to [D=256, 32768] so the DMA loads are
fully coalesced and the contraction dim lands on SBUF partitions directly.
"""

import sys

if "/opt/trn_rl_repo" not in sys.path:
    sys.path.insert(0, "/opt/trn_rl_repo")

import numpy as np

import concourse.bass as bass
import concourse.mybir as mybir
import concourse.tile as tile
from concourse import bacc
from concourse.bass_utils import run_bass_kernel_spmd

B, N, D = 64, 4096, 256
HID = D // 2  # 128
NCORES = 8
BPC = B // NCORES          # batch rows per core
TOK = BPC * N              # 32768 tokens per core
SLAB = 2048                # tokens per DMA slab (1 MiB per 128-partition load)
TS = 1024                  # tokens per compute tile (2 PSUM banks)
F32 = mybir.dt.float32
F32R = mybir.dt.float32r   # fp32 data, 1 cycle/row on the PE at free-dim >= 256

_NC = None


def _build_nc():
    nc = bacc.Bacc("TRN2", target_bir_lowering=False, debug=False)
    # matmul operands are float32r (fp32 rounded to 11 explicit mantissa
    # bits): 1 PE cycle/row instead of 4 for plain fp32. Host pre-rounds.
    sT = nc.dram_tensor("sT", [D, TOK], F32R, kind="ExternalInput")
    rn = nc.dram_tensor("rn", [BPC, N], F32, kind="ExternalInput")
    w1 = nc.dram_tensor("w1", [D, HID], F32R, kind="ExternalInput")
    b1 = nc.dram_tensor("b1", [HID, 1], F32, kind="ExternalInput")
    # layer-2 stationary per chunk c: w2d in column 32c, zeros elsewhere --
    # accumulating the slab's 4 chunks lands z at PSUM rows {0,32,64,96}
    # without PE tiling (fp32r rejects tile_position col offsets)
    w2blk = nc.dram_tensor("w2blk", [HID, 4 * 128], F32R, kind="ExternalInput")
    b2d = nc.dram_tensor("b2d", [128, 1], F32, kind="ExternalInput")
    # block-diagonal 16x16 ones: broadcasts per-row dec counts across the
    # 16-partition group holding that row in the gathered [128,256] layout
    a16 = nc.dram_tensor("a16", [128, 128], F32, kind="ExternalInput")
    dec = nc.dram_tensor("dec", [1, TOK], F32, kind="ExternalOutput")
    prob = nc.dram_tensor("prob", [1, TOK], F32, kind="ExternalOutput")

    AF = mybir.ActivationFunctionType
    ALU = mybir.AluOpType

    with tile.TileContext(nc) as tc:
        with (
            tc.tile_pool(name="consts", bufs=1) as consts,
            tc.tile_pool(name="io8", bufs=1) as io8,
            tc.tile_pool(name="sload", bufs=6) as sload,
            tc.tile_pool(name="hpool", bufs=8) as hpool,
            tc.tile_pool(name="cpool", bufs=4) as cpool,
            tc.tile_pool(name="phpool", bufs=6, space=bass.MemorySpace.PSUM) as phpool,
            tc.tile_pool(name="pzpool", bufs=2, space=bass.MemorySpace.PSUM) as pzpool,
        ):
            # slab-0 loads first: they are the long pole for compute start
            sa0 = sload.tile([128, SLAB], F32R, tag="sa")
            sb0 = sload.tile([128, SLAB], F32R, tag="sb")
            nc.sync.dma_start(sa0[:], sT[0:128, 0:SLAB])
            nc.sync.dma_start(sb0[:], sT[128:256, 0:SLAB])

            w1a = consts.tile([128, HID], F32R)
            nc.sync.dma_start(w1a[:], w1[0:128, :])
            w1b = consts.tile([128, HID], F32R)
            nc.sync.dma_start(w1b[:], w1[128:256, :])
            b1s = consts.tile([HID, 1], F32)
            nc.sync.dma_start(b1s[:], b1[:])
            w2bs = consts.tile([HID, 4 * 128], F32R)
            nc.sync.dma_start(w2bs[:], w2blk[:])
            b2s = consts.tile([128, 1], F32)
            nc.sync.dma_start(b2s[:], b2d[:])
            a16s = consts.tile([128, 128], F32)
            nc.sync.dma_start(a16s[:], a16[:])
            rns = io8.tile([BPC, N], F32)
            nc.sync.dma_start(rns[:], rn[:])

            # prob gathers into [128, 256] (token t -> partition t//256);
            # dec is derived from it in the tail (sigmoid is monotone with
            # sigmoid(0)=0.5, so prob>0.5 <=> z>0)
            probg = io8.tile([128, TOK // 128], F32)

            # rnoise argmax indicator, computed in row layout at the head
            # (vector is idle until the first slab lands), then reshaped to
            # the gathered layout for the tail fixup
            rmaxr = io8.tile([BPC, 1], F32)
            nc.vector.tensor_reduce(rmaxr[:], rns[:], mybir.AxisListType.X, ALU.max)
            fixcand = io8.tile([BPC, N], F32)
            nc.vector.tensor_scalar(fixcand[:], rns[:], rmaxr[:], None, ALU.is_equal)
            fixcandg = io8.tile([128, TOK // 128], F32)
            nc.scalar.dma_start(fixcandg[:], fixcand[:])

            # One engine per pipeline stage so no queue has head-of-line
            # blocking: tensor=matmul, vector/scalar=relu halves +
            # is_gt/sigmoid, gpsimd=dec8 move triggers, sync=input loads.
            # Layer-2 outputs for the slab's 4 chunks land at partition
            # bases 0/32/64/96 of one PSUM tile via PE tiling, so sigmoid
            # and is_gt run once per slab on [128,512] instead of per-chunk
            # on single-partition tiles.
            CH = 512  # tokens per chunk = max fp32 moving operand
            for si in range(TOK // SLAB):
                off = si * SLAB
                if si == 0:
                    sa, sb = sa0, sb0
                else:
                    sa = sload.tile([128, SLAB], F32R, tag="sa")
                    sb = sload.tile([128, SLAB], F32R, tag="sb")
                    nc.sync.dma_start(sa[:], sT[0:128, off : off + SLAB])
                    nc.sync.dma_start(sb[:], sT[128:256, off : off + SLAB])
                for hf in range(2):
                    hoff = off + hf * (SLAB // 2)
                    pzg = pzpool.tile([128, CH], F32, tag="pz")
                    phs = []
                    for c in (2 * hf, 2 * hf + 1):
                        ph = phpool.tile([128, CH], F32)
                        nc.tensor.matmul(ph[:], w1a[:],
                                         sa[:, c * CH : (c + 1) * CH],
                                         start=True, stop=False)
                        phs.append(ph)
                    hs = []
                    for c, ph in zip((2 * hf, 2 * hf + 1), phs):
                        nc.tensor.matmul(ph[:], w1b[:],
                                         sb[:, c * CH : (c + 1) * CH],
                                         start=False, stop=True)
                        h = hpool.tile([128, CH], F32R)
                        if c % 2 == 0:
                            nc.vector.tensor_scalar(h[:], ph[:],
                                                    b1s[:], 0.0, ALU.add, ALU.max)
                        else:
                            nc.scalar.activation(h[:], ph[:],
                                                 AF.Relu, bias=b1s[:])
                        hs.append(h)
                    for k, h in enumerate(hs):
                        nc.tensor.matmul(pzg[:], w2bs[:, 128 * k : 128 * (k + 1)],
                                         h[:], start=(k == 0), stop=(k == 1))
                    pcg = cpool.tile([64, CH], F32, tag="pc")
                    nc.scalar.activation(pcg[:], pzg[0:64, :],
                                         AF.Sigmoid, bias=b2s[0:64, :])
                    # strided-partition source {0,32} -> one DMA per output
                    nc.scalar.dma_start(prob[0:1, hoff : hoff + SLAB // 2],
                                        pcg[0:64:32, :])
                    nc.scalar.dma_start(
                        probg[4 * (2 * si + hf) : 4 * (2 * si + hf) + 4, :],
                        pcg[0:64:32, :])

            # Row correction: rows with no active slot get argmax(rnoise)
            # forced on. All in the [128,256] layout: per-partition counts,
            # then the block-diagonal-ones matmul broadcasts each row's
            # total count to its 16 partitions (sum==0 <=> max==0).
            dec8g = io8.tile([128, TOK // 128], F32)
            nc.vector.tensor_scalar(dec8g[:], probg[:], 0.5, None, ALU.is_gt)
            pcnt = pzpool.tile([128, CH], F32, tag="pz")
            nc.tensor.matmul(pcnt[:, 0 : TOK // 128], a16s[:],
                             dec8g[:], start=True, stop=True)
            cntg = io8.tile([128, 1], F32)
            nc.vector.tensor_reduce(cntg[:], pcnt[:, 0 : TOK // 128],
                                    mybir.AxisListType.X, ALU.add)
            needg = io8.tile([128, 1], F32)
            nc.vector.tensor_scalar(needg[:], cntg[:], 0.0, None, ALU.is_equal)
            nc.vector.tensor_scalar(fixcandg[:], fixcandg[:], needg[:], None, ALU.mult)
            nc.vector.tensor_max(dec8g[:], dec8g[:], fixcandg[:])
            nc.scalar.dma_start(dec[0:1, :], dec8g[:])

    nc.compile()
    return nc


def _get_nc():
    global _NC
    if _NC is None:
        _NC = _build_nc()
    return _NC


def _round_fp32r(x):
    # round-to-nearest-even at mantissa bit 12 (matches HW fp32_to_fp32r)
    b = np.ascontiguousarray(x, dtype=np.float32).view(np.uint32)
    r = (b + np.uint32(0x7FF) + ((b >> np.uint32(12)) & np.uint32(1))) & np.uint32(
        0xFFFFF000
    )
    return r.view(np.float32)


def _make_in_maps(s, W1, b1, W2, b2, rnoise):
    s = _round_fp32r(s)
    w1 = _round_fp32r(W1)
    b1c = np.ascontiguousarray(b1, dtype=np.float32).reshape(HID, 1)
    w2dc = _round_fp32r(np.asarray(W2[:, 1] - W2[:, 0], dtype=np.float32))
    w2blkc = np.zeros((HID, 4 * 128), dtype=np.float32)
    for c in range(4):
        w2blkc[:, 128 * c + 32 * c] = w2dc
    a16c = (
        (np.arange(128)[:, None] // 16) == (np.arange(128)[None, :] // 16)
    ).astype(np.float32)
    b2dv = np.float32(b2[1] - b2[0])
    b2dc = np.full((128, 1), b2dv, dtype=np.float32)
    rn = np.ascontiguousarray(rnoise, dtype=np.float32)

    # [NCORES, D, TOK] with the contraction dim outer -> coalesced loads
    sT = np.ascontiguousarray(
        s.reshape(NCORES, TOK, D).transpose(0, 2, 1)
    )
    return [
        {
            "sT": sT[c],
            "rn": rn.reshape(NCORES, BPC, N)[c],
            "w1": w1,
            "b1": b1c,
            "w2blk": w2blkc,
            "b2d": b2dc,
            "a16": a16c,
        }
        for c in range(NCORES)
    ]


def run(s, W1, b1, W2, b2, rnoise, trace=False):
    nc = _get_nc()
    in_maps = _make_in_maps(s, W1, b1, W2, b2, rnoise)
    res = run_bass_kernel_spmd(nc, in_maps, list(range(NCORES)), trace=trace)
    dec = np.concatenate(
        [r["dec"].reshape(BPC, N) for r in res.results], axis=0
    )
    prob = np.concatenate(
        [r["prob"].reshape(BPC, N) for r in res.results], axis=0
    )
    return (dec, prob), res


def kernel(s, W1, b1, W2, b2, rnoise):
    (dec, prob), _ = run(s, W1, b1, W2, b2, rnoise)
    return dec, prob

